# revision 43
# baseline (speedup 1.0000x reference)
"""Trainium2 Bass kernel for nn_AggregateJoint (grouped 2-layer MLP over parts).

Math: for each of R = b*f rows (x transposed to [R, n=64]), 16 parts each take
4 contiguous channels -> Linear(4,16) -> LeakyReLU -> Linear(16,3) -> BatchNorm
(running stats) -> LeakyReLU -> concat to 48 channels -> output [b, 48, f].

Mapping: per batch b_idx, X = x[b_idx] is [64, 512] (rows=channels, cols=f).
Both layers are block-diagonal matmuls over columns:
  stage 1: h = leaky(A1 @ X + b1), A1 block-diag [256, 64] (16 blocks 16x4)
  stage 2: y = leaky(A2 @ h + c2), A2 block-diag [48, 256] (BN scale folded)

Per NeuronCore (8-way batch-parallel, 32 batches each), per pair of batches:
  - stage 1: 4 concurrent row-tiled fp32r matmuls (K=32, tile_position (32i,0))
    -> PSUM: one [128, 1024] tile (part-group a) + two [128, 512] tiles (group
    b; finer slots so the slower DVE epilogue returns banks incrementally)
  - epilogue 1: leaky+bias PSUM->SBUF fp16 h (fp16 keeps 10 mantissa bits and
    enables stage-2 col tiling): ACT Lrelu(alpha) one wide op for group a, a
    custom one-pass leaky DVE op per batch for group b. ACT+DVE are the
    bottleneck engines (~27us each per core).
  - stage 2: 4 concurrent col-tiled fp16 matmuls (M=24, tile_position (0,32j))
    -> PSUM [128, 512] rows 0-23/32-55/64-87/96-119
  - epilogue 2: leaky+bias -> an SBUF [128, 4, 512] tile covering 4 pairs,
    mostly on ACT
  - input DMA: 512KB per two pairs on the sync (HWDGE) ring; output DMA: four
    192KB contiguous-partition block DMAs per 4 pairs on the gpsimd ring (the
    final quad goes via sync to avoid the SWDGE completion-drain at the tail).
    Multi-level partition-dim DMA access patterns are silently broken on this
    stack -- only single contiguous partition blocks are used.
"""
import os
import sys
import types

import numpy as np

P, IN, H, O = 16, 4, 16, 3
NEG = 0.01
BN_EPS = 1e-5
NCORES = 8
B, N, F = 256, 64, 512
BPC = B // NCORES          # batches per core
PAIRS = BPC // 2

_cache = {}


def _install_ntff_hook():
    """antenv.axon_hooks is absent in this image; recreate it and register the
    ctypes NTFF hook so trace=True works (used by test.py, harmless otherwise)."""
    import antenv

    if "antenv.axon_hooks" in sys.modules:
        return
    m = types.ModuleType("antenv.axon_hooks")
    m._hook = None
    m.set_axon_ntff_profile_hook = lambda h: setattr(m, "_hook", h)
    m.get_axon_ntff_profile_hook = lambda: m._hook
    sys.modules["antenv.axon_hooks"] = m
    antenv.axon_hooks = m
    try:
        from trn_agent_boot.trn_boot import _ntff_profile_via_ctypes

        m._hook = _ntff_profile_via_ctypes("/opt/axon/libaxon_pjrt.so")
    except Exception:
        pass


def _register_leaky():
    """Custom DVE op: out = relu(in0 + s0)*s1 + (in0 + s0)*imm2.
    With s1=0.99, imm2=0.01 this is leaky_relu(in0 + bias) in one pass."""
    import concourse.dve_ops as dve_ops
    from concourse.dve_spec import Spec, Src0, C0, C1, C2, relu, lower
    from concourse.dve_uop import DveOpSpec

    name = "LEAKY_BIAS_ANT"
    if name in dve_ops._SUB_OPCODE_FOR_NAME:
        return next(op for op in dve_ops.OPS if op.name == name)

    def ref(in0, in1, s0, s1, imm2):
        z = in0.astype(np.float32) + s0
        zc = np.nan_to_num(z, nan=0.0, posinf=np.inf, neginf=-np.inf)
        return np.maximum(zc, 0) * s1 + z * imm2

    t = Src0 + C0
    spec = Spec(body=relu(t) * C1 + t * C2, reference=ref)
    row = dve_ops._CUSTOM_DVE_ROW_BASE + len(dve_ops.OPS)
    shas = {}
    for ver in ("v3", "v4"):
        uops = lower(spec, ver=ver)
        shas[ver] = DveOpSpec(name=name, opcode=row, uops=uops, rd1_en=False).sha(ver)
    op = dve_ops.DveOp(name, spec, subdim=False, uops_sha=shas)
    dve_ops.OPS.append(op)
    dve_ops.CUSTOM_DVE_SPECS[name] = spec
    dve_ops._SUB_OPCODE_FOR_NAME[name] = row
    return op


def _prep_weights(parts, W1, b1, W2, b2, gamma, beta, mean, var):
    """Host-side packing of the tiny per-part weights into SBUF layouts."""
    parts = np.asarray(parts)
    assert np.array_equal(parts.ravel(), np.arange(N)), "non-contiguous parts"
    s = (gamma / np.sqrt(var + BN_EPS)).astype(np.float32)          # [P, O]
    c2 = ((b2 - mean) * s + beta).astype(np.float32)                # [P, O]

    # stage-1 lhsT blocks [32, 128]: rows = in-ch local, cols = h-ch (16p+j).
    # SBUF rows: 0-31 grp a, 32-63 grp b, 64-95 grp a, 96-127 grp b (one copy
    # per PE row-group so two batches run concurrently).
    blk_a = np.zeros((32, 128), np.float32)
    blk_b = np.zeros((32, 128), np.float32)
    for pl in range(8):
        blk_a[4 * pl:4 * pl + 4, 16 * pl:16 * pl + 16] = W1[pl]
        blk_b[4 * pl:4 * pl + 4, 16 * pl:16 * pl + 16] = W1[8 + pl]
    w1sb = np.concatenate([blk_a, blk_b, blk_a, blk_b], axis=0)     # [128, 128]

    # stage-2 lhsT [128, 48] in fp16: cols 0-23 grp a block, 24-47 grp b block
    w2s = (W2 * s[:, None, :]).astype(np.float32)                   # [P, H, O]
    w2sb = np.zeros((128, 48), np.float32)
    for pl in range(8):
        w2sb[16 * pl:16 * pl + 16, 3 * pl:3 * pl + 3] = w2s[pl]
        w2sb[16 * pl:16 * pl + 16, 24 + 3 * pl:24 + 3 * pl + 3] = w2s[8 + pl]
    w2sb = w2sb.astype(np.float16)

    b1a = np.zeros((128, 1), np.float32)
    b1b = np.zeros((128, 1), np.float32)
    for pl in range(8):
        b1a[16 * pl:16 * pl + 16, 0] = b1[pl]
        b1b[16 * pl:16 * pl + 16, 0] = b1[8 + pl]
    # ep2 bias on PSUM partition layout: 0-23 c2a, 32-55 c2b, 64-87 c2a,
    # 96-119 c2b (batch-even / batch-odd share values)
    b2v = np.zeros((128, 1), np.float32)
    ca = c2[0:8].reshape(24)
    cb = c2[8:16].reshape(24)
    b2v[0:24, 0] = ca
    b2v[32:56, 0] = cb
    b2v[64:88, 0] = ca
    b2v[96:120, 0] = cb
    wq = np.zeros((128, 27), np.float32)
    w2u = w2sb.view(np.uint16)                    # [128, 48]
    wq.view(np.uint32)[:, 0:24] = (w2u[:, 1::2].astype(np.uint32) << 16) | w2u[:, 0::2].astype(np.uint32)
    wq[:, 24:25] = b1a
    wq[:, 25:26] = b1b
    wq[:, 26:27] = b2v
    return w1sb, wq


def _build():
    _install_ntff_hook()
    LEAKY = _register_leaky()

    from contextlib import ExitStack

    import concourse.bacc as bacc
    import concourse.tile as tile
    import concourse.mybir as mybir
    from concourse.vector_clock import ScopedClock

    if os.environ.get("KTAIL", "light") == "light" and not getattr(tile.TileContext, "_tail_patched", False):
        # Single-shot NEFF: skip the exit-time semaphore clears and the second
        # all-engine barrier (the preamble re-initialises semaphores each run).
        def _light_drain(self, tick_clock, wait_clock):
            drain_inst = self.nc.sync.drain()
            wait_clock.add_sem_waits(
                drain_inst.ins, ScopedClock({None: tick_clock.global_clock}))
            if os.environ.get("KTAIL2", "nobarrier") != "nobarrier":
                self.nc.all_engine_barrier()
            popped = self.nc._tile_sem_poison_stack.pop()
            assert popped is self._sem_poison

        tile.TileContext._drain_and_barrier = _light_drain
        tile.TileContext._tail_patched = True

    f32 = mybir.dt.float32
    f32r = mybir.dt.float32r
    f16 = mybir.dt.float16
    AF = mybir.ActivationFunctionType

    nc = bacc.Bacc("TRN2", target_bir_lowering=False, debug=False)

    x_d = nc.dram_tensor("x", [BPC, N, F], f32r, kind="ExternalInput").ap()
    wp_d = nc.dram_tensor("wp", [128, 128], f32r, kind="ExternalInput").ap()
    wq_d = nc.dram_tensor("wq", [128, 27], f32, kind="ExternalInput").ap()
    y_d = nc.dram_tensor("y", [BPC, 48, F], f32, kind="ExternalOutput").ap()

    # x as [quads u][pair q][128 rows][512]:  u in 0..7, q in 0..1
    x_q = x_d.rearrange("(u q two) n f -> u (two n) q f", q=2, two=2)  # [8,128,2,512]
    # y as [quad-of-pairs U][parity q][grp g][c 24][pair-in-quad m][f]
    y_quads = y_d.rearrange("(u m q) (g c) f -> u q g c m f", m=4, q=2, g=2)

    with tile.TileContext(nc) as tc, ExitStack() as ctx:
        singles = ctx.enter_context(tc.tile_pool(name="singles", bufs=1))
        xp = ctx.enter_context(tc.tile_pool(name="xp", bufs=5))
        hsb = ctx.enter_context(tc.tile_pool(name="hsb", bufs=6))
        osb = ctx.enter_context(tc.tile_pool(name="osb", bufs=4))
        hpa = ctx.enter_context(tc.tile_pool(name="hpa", bufs=2, space="PSUM"))
        hpb = ctx.enter_context(tc.tile_pool(name="hpb", bufs=2, space="PSUM"))
        ops = ctx.enter_context(tc.tile_pool(name="ops", bufs=1, space="PSUM"))

        wp_sb = singles.tile([128, 128], f32r)
        wq_sb = singles.tile([128, 27], f32)
        nc.sync.dma_start(out=wp_sb, in_=wp_d)
        nc.sync.dma_start(out=wq_sb, in_=wq_d)
        w1_sb = wp_sb
        w2_sb = wq_sb[:, 0:24].bitcast(f16)
        b1a_sb = wq_sb[:, 24:25]
        b1b_sb = wq_sb[:, 25:26]
        b2_sb = wq_sb[:, 26:27]

        LAG = 1  # stage-2 trails stage-1 by one pair (denser PE stream)
        xtiles = {}
        htiles = {}
        otile = [None]
        pso_ref = [None]

        def emit_stage1(t):
            u, q = divmod(t, 2)
            if q == 0:
                x_sb = xp.tile([128, 2, 512], f32r, tag="x")
                nc.sync.dma_start(out=x_sb, in_=x_q[u])
                xtiles[u] = x_sb
            else:
                x_sb = xtiles[u]

            ps_a = hpa.tile([128, 1024], f32, tag="ha")
            ps_be = hpb.tile([128, 512], f32, tag="hb")
            ps_bo = hpb.tile([128, 512], f32, tag="hb")
            nc.tensor.matmul(ps_a[:, 0:512], w1_sb[0:32, :], x_sb[0:32, q, :],
                             start=True, stop=True, tile_position=(0, 0))
            nc.tensor.matmul(ps_be, w1_sb[32:64, :], x_sb[32:64, q, :],
                             start=True, stop=True, tile_position=(32, 0))
            nc.tensor.matmul(ps_a[:, 512:1024], w1_sb[64:96, :], x_sb[64:96, q, :],
                             start=True, stop=True, tile_position=(64, 0))
            nc.tensor.matmul(ps_bo, w1_sb[96:128, :], x_sb[96:128, q, :],
                             start=True, stop=True, tile_position=(96, 0))

            h_a = hsb.tile([128, 1024], f16, tag="ha")
            h_b = hsb.tile([128, 1024], f16, tag="hb")
            # ACT: one wide op for part-group a; DVE: per-batch ops for group b
            nc.scalar.activation(h_a, ps_a, AF.Lrelu, bias=b1a_sb,
                                 scale=1.0, alpha=NEG)
            nc.vector._custom_dve(LEAKY, out=h_b[:, 0:512], in0=ps_be, s0=b1b_sb,
                                  s1=1.0 - NEG, imm2=NEG)
            nc.vector._custom_dve(LEAKY, out=h_b[:, 512:1024], in0=ps_bo, s0=b1b_sb,
                                  s1=1.0 - NEG, imm2=NEG)
            htiles[t] = (h_a, h_b)

        def emit_stage2(t):
            h_a, h_b = htiles.pop(t)
            if t % 2 == 0:
                ps_o_new = ops.tile([128, 1024], f32, tag="o")
                pso_ref[0] = ps_o_new
            ps_o = pso_ref[0]
            off = (t % 2) * 512
            nc.tensor.matmul(ps_o[0:24, off:off + 512], w2_sb[:, 0:24], h_a[:, 0:512],
                             start=True, stop=True, tile_position=(0, 0))
            nc.tensor.matmul(ps_o[32:56, off:off + 512], w2_sb[:, 24:48], h_b[:, 0:512],
                             start=True, stop=True, tile_position=(0, 32))
            nc.tensor.matmul(ps_o[64:88, off:off + 512], w2_sb[:, 0:24], h_a[:, 512:1024],
                             start=True, stop=True, tile_position=(0, 64))
            nc.tensor.matmul(ps_o[96:120, off:off + 512], w2_sb[:, 24:48], h_b[:, 512:1024],
                             start=True, stop=True, tile_position=(0, 96))

            m = t % 4
            if m == 0:
                o_new = osb.tile([128, 4, 512], f32, tag="out")
                otile[0] = o_new
            o_sb = otile[0]
            if t % 2 == 1:
                # one [120, 1024] epilogue covers both pairs of this psum tile
                dst = o_sb[0:120, m - 1:m + 1, :]
                srcv = ps_o[0:120, :].rearrange("p (mm f) -> p mm f", mm=2)
                if t != 15:
                    nc.scalar.activation(dst, srcv, AF.Lrelu,
                                         bias=b2_sb[0:120], scale=1.0, alpha=NEG)
                else:
                    nc.vector._custom_dve(LEAKY, out=dst, in0=srcv,
                                          s0=b2_sb[0:120], s1=1.0 - NEG, imm2=NEG)

            if m == 3:
                # 4 contiguous-partition block DMAs cover the last 4 pairs
                U = t // 4
                o_blocks = o_sb.rearrange("(gg c) m f -> gg c m f", gg=4)[:, 0:24, :, :]
                last = U == (PAIRS // 4 - 1)
                for blk in range(4):
                    q, g = divmod(blk, 2)
                    eng = (nc.sync if blk >= 2 else nc.gpsimd) if last else nc.gpsimd
                    # dst: batches 8U+2m+q (m=0..3), channels 24g..24g+24
                    eng.dma_start(
                        out=y_quads[U, q, g],        # [24, 4, 512]
                        in_=o_blocks[blk])

        for t in range(PAIRS + LAG):
            if t < PAIRS:
                emit_stage1(t)
            if t >= LAG:
                emit_stage2(t - LAG)

    nc.compile()
    return nc


def kernel(**inputs):
    import concourse.bass_utils as bass_utils

    if "nc" not in _cache:
        _cache["nc"] = _build()
        bass_utils.upload_artifacts = lambda tmpdir: "local://" + tmpdir
    nc = _cache["nc"]

    x = np.ascontiguousarray(np.asarray(inputs["x"], dtype=np.float32))
    w1sb, wq = _prep_weights(
        inputs["parts"],
        np.asarray(inputs["W1"], np.float32), np.asarray(inputs["b1"], np.float32),
        np.asarray(inputs["W2"], np.float32), np.asarray(inputs["b2"], np.float32),
        np.asarray(inputs["gamma"], np.float32), np.asarray(inputs["beta"], np.float32),
        np.asarray(inputs["mean"], np.float32), np.asarray(inputs["var"], np.float32),
    )

    in_maps = []
    for c in range(NCORES):
        in_maps.append({"x": x[c * BPC:(c + 1) * BPC], "wp": w1sb, "wq": wq})

    trace = bool(os.environ.get("KERNEL_TRACE"))
    kw = {}
    if trace:
        kw = dict(trace=True, trace_cores=[0], tmpdir=os.environ.get("KERNEL_TRACE_DIR"))
    res = bass_utils.run_bass_kernel_spmd(
        nc, in_maps, core_ids=list(range(NCORES)), **kw)
    _cache["last_result"] = res

    out = np.concatenate([r["y"] for r in res.results], axis=0)  # [256, 48, 512]
    return out


# revision 44
# speedup vs baseline: 1.0425x; 1.0425x over previous
"""Trainium2 Bass kernel for nn_AggregateJoint (grouped 2-layer MLP over parts).

Math: for each of R = b*f rows (x transposed to [R, n=64]), 16 parts each take
4 contiguous channels -> Linear(4,16) -> LeakyReLU -> Linear(16,3) -> BatchNorm
(running stats) -> LeakyReLU -> concat to 48 channels -> output [b, 48, f].

Mapping: per batch b_idx, X = x[b_idx] is [64, 512] (rows=channels, cols=f).
Both layers are block-diagonal matmuls over columns:
  stage 1: h = leaky(A1 @ X + b1), A1 block-diag [256, 64] (16 blocks 16x4)
  stage 2: y = leaky(A2 @ h + c2), A2 block-diag [48, 256] (BN scale folded)

Per NeuronCore (8-way batch-parallel, 32 batches each), per pair of batches:
  - stage 1: 4 concurrent row-tiled fp32r matmuls (K=32, tile_position (32i,0))
    -> PSUM: one [128, 1024] tile (part-group a) + two [128, 512] tiles (group
    b; finer slots so the slower DVE epilogue returns banks incrementally)
  - epilogue 1: leaky+bias PSUM->SBUF fp16 h (fp16 keeps 10 mantissa bits and
    enables stage-2 col tiling): ACT Lrelu(alpha) one wide op for group a, a
    custom one-pass leaky DVE op per batch for group b. ACT+DVE are the
    bottleneck engines (~27us each per core).
  - stage 2: 4 concurrent col-tiled fp16 matmuls (M=24, tile_position (0,32j))
    -> PSUM [128, 512] rows 0-23/32-55/64-87/96-119
  - epilogue 2: leaky+bias -> an SBUF [128, 4, 512] tile covering 4 pairs,
    mostly on ACT
  - input DMA: 512KB per two pairs on the sync (HWDGE) ring; output DMA: four
    192KB contiguous-partition block DMAs per 4 pairs on the gpsimd ring (the
    final quad goes via sync to avoid the SWDGE completion-drain at the tail).
    Multi-level partition-dim DMA access patterns are silently broken on this
    stack -- only single contiguous partition blocks are used.
"""
import os
import sys
import types

import numpy as np

P, IN, H, O = 16, 4, 16, 3
NEG = 0.01
BN_EPS = 1e-5
NCORES = 8
B, N, F = 256, 64, 512
BPC = B // NCORES          # batches per core
PAIRS = BPC // 2

_cache = {}


def _install_ntff_hook():
    """antenv.axon_hooks is absent in this image; recreate it and register the
    ctypes NTFF hook so trace=True works (used by test.py, harmless otherwise)."""
    import antenv

    if "antenv.axon_hooks" in sys.modules:
        return
    m = types.ModuleType("antenv.axon_hooks")
    m._hook = None
    m.set_axon_ntff_profile_hook = lambda h: setattr(m, "_hook", h)
    m.get_axon_ntff_profile_hook = lambda: m._hook
    sys.modules["antenv.axon_hooks"] = m
    antenv.axon_hooks = m
    try:
        from trn_agent_boot.trn_boot import _ntff_profile_via_ctypes

        m._hook = _ntff_profile_via_ctypes("/opt/axon/libaxon_pjrt.so")
    except Exception:
        pass


def _register_leaky():
    """Custom DVE op: out = relu(in0 + s0)*s1 + (in0 + s0)*imm2.
    With s1=0.99, imm2=0.01 this is leaky_relu(in0 + bias) in one pass."""
    import concourse.dve_ops as dve_ops
    from concourse.dve_spec import Spec, Src0, C0, C1, C2, relu, lower
    from concourse.dve_uop import DveOpSpec

    name = "LEAKY_BIAS_ANT"
    if name in dve_ops._SUB_OPCODE_FOR_NAME:
        return next(op for op in dve_ops.OPS if op.name == name)

    def ref(in0, in1, s0, s1, imm2):
        z = in0.astype(np.float32) + s0
        zc = np.nan_to_num(z, nan=0.0, posinf=np.inf, neginf=-np.inf)
        return np.maximum(zc, 0) * s1 + z * imm2

    t = Src0 + C0
    spec = Spec(body=relu(t) * C1 + t * C2, reference=ref)
    row = dve_ops._CUSTOM_DVE_ROW_BASE + len(dve_ops.OPS)
    shas = {}
    for ver in ("v3", "v4"):
        uops = lower(spec, ver=ver)
        shas[ver] = DveOpSpec(name=name, opcode=row, uops=uops, rd1_en=False).sha(ver)
    op = dve_ops.DveOp(name, spec, subdim=False, uops_sha=shas)
    dve_ops.OPS.append(op)
    dve_ops.CUSTOM_DVE_SPECS[name] = spec
    dve_ops._SUB_OPCODE_FOR_NAME[name] = row
    return op


def _prep_weights(parts, W1, b1, W2, b2, gamma, beta, mean, var):
    """Host-side packing of the tiny per-part weights into SBUF layouts."""
    parts = np.asarray(parts)
    assert np.array_equal(parts.ravel(), np.arange(N)), "non-contiguous parts"
    s = (gamma / np.sqrt(var + BN_EPS)).astype(np.float32)          # [P, O]
    c2 = ((b2 - mean) * s + beta).astype(np.float32)                # [P, O]

    # stage-1 lhsT blocks [32, 128]: rows = in-ch local, cols = h-ch (16p+j).
    # SBUF rows: 0-31 grp a, 32-63 grp b, 64-95 grp a, 96-127 grp b (one copy
    # per PE row-group so two batches run concurrently).
    blk_a = np.zeros((32, 128), np.float32)
    blk_b = np.zeros((32, 128), np.float32)
    for pl in range(8):
        blk_a[4 * pl:4 * pl + 4, 16 * pl:16 * pl + 16] = W1[pl]
        blk_b[4 * pl:4 * pl + 4, 16 * pl:16 * pl + 16] = W1[8 + pl]
    w1sb = np.concatenate([blk_a, blk_b, blk_a, blk_b], axis=0)     # [128, 128]

    # stage-2 lhsT [128, 48] in fp16: cols 0-23 grp a block, 24-47 grp b block
    w2s = (W2 * s[:, None, :]).astype(np.float32)                   # [P, H, O]
    w2sb = np.zeros((128, 48), np.float32)
    for pl in range(8):
        w2sb[16 * pl:16 * pl + 16, 3 * pl:3 * pl + 3] = w2s[pl]
        w2sb[16 * pl:16 * pl + 16, 24 + 3 * pl:24 + 3 * pl + 3] = w2s[8 + pl]
    w2sb = w2sb.astype(np.float16)

    b1a = np.zeros((128, 1), np.float32)
    b1b = np.zeros((128, 1), np.float32)
    for pl in range(8):
        b1a[16 * pl:16 * pl + 16, 0] = b1[pl]
        b1b[16 * pl:16 * pl + 16, 0] = b1[8 + pl]
    # ep2 bias on PSUM partition layout: 0-23 c2a, 32-55 c2b, 64-87 c2a,
    # 96-119 c2b (batch-even / batch-odd share values)
    b2v = np.zeros((128, 1), np.float32)
    ca = c2[0:8].reshape(24)
    cb = c2[8:16].reshape(24)
    b2v[0:24, 0] = ca
    b2v[32:56, 0] = cb
    b2v[64:88, 0] = ca
    b2v[96:120, 0] = cb
    wq = np.zeros((128, 27), np.float32)
    w2u = w2sb.view(np.uint16)                    # [128, 48]
    wq.view(np.uint32)[:, 0:24] = (w2u[:, 1::2].astype(np.uint32) << 16) | w2u[:, 0::2].astype(np.uint32)
    wq[:, 24:25] = b1a
    wq[:, 25:26] = b1b
    wq[:, 26:27] = b2v
    return w1sb, wq


def _build():
    _install_ntff_hook()
    LEAKY = _register_leaky()

    from contextlib import ExitStack

    import concourse.bacc as bacc
    import concourse.tile as tile
    import concourse.mybir as mybir
    from concourse.vector_clock import ScopedClock

    if os.environ.get("KTAIL", "light") == "light" and not getattr(tile.TileContext, "_tail_patched", False):
        # Single-shot NEFF: skip the exit-time semaphore clears and the second
        # all-engine barrier (the preamble re-initialises semaphores each run).
        def _light_drain(self, tick_clock, wait_clock):
            drain_inst = self.nc.sync.drain()
            wait_clock.add_sem_waits(
                drain_inst.ins, ScopedClock({None: tick_clock.global_clock}))
            if os.environ.get("KTAIL2", "nobarrier") != "nobarrier":
                self.nc.all_engine_barrier()
            popped = self.nc._tile_sem_poison_stack.pop()
            assert popped is self._sem_poison

        tile.TileContext._drain_and_barrier = _light_drain
        tile.TileContext._tail_patched = True

    f32 = mybir.dt.float32
    f32r = mybir.dt.float32r
    f16 = mybir.dt.float16
    AF = mybir.ActivationFunctionType

    nc = bacc.Bacc("TRN2", target_bir_lowering=False, debug=False)

    x_d = nc.dram_tensor("x", [BPC, N, F], f32r, kind="ExternalInput").ap()
    wp_d = nc.dram_tensor("wp", [128, 128], f32r, kind="ExternalInput").ap()
    wq_d = nc.dram_tensor("wq", [128, 27], f32, kind="ExternalInput").ap()
    y_d = nc.dram_tensor("y", [BPC, 48, F], f32, kind="ExternalOutput").ap()

    # x as [quads u][pair q][128 rows][512]:  u in 0..7, q in 0..1
    x_q = x_d.rearrange("(u q two) n f -> u (two n) q f", q=2, two=2)  # [8,128,2,512]
    # y as [quad-of-pairs U][parity q][grp g][c 24][pair-in-quad m][f]
    y_quads = y_d.rearrange("(u m q) (g c) f -> u q g c m f", m=4, q=2, g=2)

    with tile.TileContext(nc) as tc, ExitStack() as ctx:
        singles = ctx.enter_context(tc.tile_pool(name="singles", bufs=1))
        xp = ctx.enter_context(tc.tile_pool(name="xp", bufs=5))
        hsb = ctx.enter_context(tc.tile_pool(name="hsb", bufs=6))
        osb = ctx.enter_context(tc.tile_pool(name="osb", bufs=4))
        hpa = ctx.enter_context(tc.tile_pool(name="hpa", bufs=2, space="PSUM"))
        hpb = ctx.enter_context(tc.tile_pool(name="hpb", bufs=3, space="PSUM"))
        ops = ctx.enter_context(tc.tile_pool(name="ops", bufs=1, space="PSUM"))

        wp_sb = singles.tile([128, 128], f32r)
        wq_sb = singles.tile([128, 27], f32)
        nc.sync.dma_start(out=wp_sb, in_=wp_d)
        nc.sync.dma_start(out=wq_sb, in_=wq_d)
        w1_sb = wp_sb
        w2_sb = wq_sb[:, 0:24].bitcast(f16)
        b1a_sb = wq_sb[:, 24:25]
        b1b_sb = wq_sb[:, 25:26]
        b2_sb = wq_sb[:, 26:27]

        LAG = 1  # stage-2 trails stage-1 by one pair (denser PE stream)
        xtiles = {}
        htiles = {}
        otile = [None]

        def emit_stage1(t):
            u, q = divmod(t, 2)
            if q == 0:
                x_sb = xp.tile([128, 2, 512], f32r, tag="x")
                if t == 0:
                    # split the first load so pair 0 unblocks ~1us earlier
                    nc.sync.dma_start(out=x_sb[:, 0, :], in_=x_q[u][:, 0, :])
                    nc.sync.dma_start(out=x_sb[:, 1, :], in_=x_q[u][:, 1, :])
                else:
                    nc.sync.dma_start(out=x_sb, in_=x_q[u])
                xtiles[u] = x_sb
            else:
                x_sb = xtiles[u]

            ps_a = hpa.tile([128, 1024], f32, tag="ha")
            ps_be = hpb.tile([128, 512], f32, tag="hb")
            ps_bo = hpb.tile([128, 512], f32, tag="hb")
            nc.tensor.matmul(ps_a[:, 0:512], w1_sb[0:32, :], x_sb[0:32, q, :],
                             start=True, stop=True, tile_position=(0, 0))
            nc.tensor.matmul(ps_be, w1_sb[32:64, :], x_sb[32:64, q, :],
                             start=True, stop=True, tile_position=(32, 0))
            nc.tensor.matmul(ps_a[:, 512:1024], w1_sb[64:96, :], x_sb[64:96, q, :],
                             start=True, stop=True, tile_position=(64, 0))
            nc.tensor.matmul(ps_bo, w1_sb[96:128, :], x_sb[96:128, q, :],
                             start=True, stop=True, tile_position=(96, 0))

            h_a = hsb.tile([128, 1024], f16, tag="ha")
            h_b = hsb.tile([128, 1024], f16, tag="hb")
            # ACT: one wide op for part-group a; DVE: per-batch ops for group b
            nc.scalar.activation(h_a, ps_a, AF.Lrelu, bias=b1a_sb,
                                 scale=1.0, alpha=NEG)
            nc.vector._custom_dve(LEAKY, out=h_b[:, 0:512], in0=ps_be, s0=b1b_sb,
                                  s1=1.0 - NEG, imm2=NEG)
            nc.vector._custom_dve(LEAKY, out=h_b[:, 512:1024], in0=ps_bo, s0=b1b_sb,
                                  s1=1.0 - NEG, imm2=NEG)
            htiles[t] = (h_a, h_b)

        def emit_stage2(t):
            h_a, h_b = htiles.pop(t)
            ps_o = ops.tile([128, 512], f32, tag="o")
            nc.tensor.matmul(ps_o[0:24, :], w2_sb[:, 0:24], h_a[:, 0:512],
                             start=True, stop=True, tile_position=(0, 0))
            nc.tensor.matmul(ps_o[32:56, :], w2_sb[:, 24:48], h_b[:, 0:512],
                             start=True, stop=True, tile_position=(0, 32))
            nc.tensor.matmul(ps_o[64:88, :], w2_sb[:, 0:24], h_a[:, 512:1024],
                             start=True, stop=True, tile_position=(0, 64))
            nc.tensor.matmul(ps_o[96:120, :], w2_sb[:, 24:48], h_b[:, 512:1024],
                             start=True, stop=True, tile_position=(0, 96))

            m = t % 4
            if m == 0:
                o_new = osb.tile([128, 4, 512], f32, tag="out")
                otile[0] = o_new
            o_sb = otile[0]
            if t % 8 != 7:
                nc.scalar.activation(o_sb[0:120, m, :], ps_o[0:120, :], AF.Lrelu,
                                     bias=b2_sb[0:120], scale=1.0, alpha=NEG)
            else:
                nc.vector._custom_dve(LEAKY, out=o_sb[0:120, m, :], in0=ps_o[0:120, :],
                                      s0=b2_sb[0:120], s1=1.0 - NEG, imm2=NEG)

            if m == 3:
                # 4 contiguous-partition block DMAs cover the last 4 pairs
                U = t // 4
                o_blocks = o_sb.rearrange("(gg c) m f -> gg c m f", gg=4)[:, 0:24, :, :]
                last = U == (PAIRS // 4 - 1)
                for blk in range(4):
                    q, g = divmod(blk, 2)
                    eng = (nc.sync if blk >= 2 else nc.gpsimd) if last else nc.gpsimd
                    # dst: batches 8U+2m+q (m=0..3), channels 24g..24g+24
                    eng.dma_start(
                        out=y_quads[U, q, g],        # [24, 4, 512]
                        in_=o_blocks[blk])

        for t in range(PAIRS + LAG):
            if t < PAIRS:
                emit_stage1(t)
            if t >= LAG:
                emit_stage2(t - LAG)

    nc.compile()
    return nc


def kernel(**inputs):
    import concourse.bass_utils as bass_utils

    if "nc" not in _cache:
        _cache["nc"] = _build()
        bass_utils.upload_artifacts = lambda tmpdir: "local://" + tmpdir
    nc = _cache["nc"]

    x = np.ascontiguousarray(np.asarray(inputs["x"], dtype=np.float32))
    w1sb, wq = _prep_weights(
        inputs["parts"],
        np.asarray(inputs["W1"], np.float32), np.asarray(inputs["b1"], np.float32),
        np.asarray(inputs["W2"], np.float32), np.asarray(inputs["b2"], np.float32),
        np.asarray(inputs["gamma"], np.float32), np.asarray(inputs["beta"], np.float32),
        np.asarray(inputs["mean"], np.float32), np.asarray(inputs["var"], np.float32),
    )

    in_maps = []
    for c in range(NCORES):
        in_maps.append({"x": x[c * BPC:(c + 1) * BPC], "wp": w1sb, "wq": wq})

    trace = bool(os.environ.get("KERNEL_TRACE"))
    kw = {}
    if trace:
        kw = dict(trace=True, trace_cores=[0], tmpdir=os.environ.get("KERNEL_TRACE_DIR"))
    res = bass_utils.run_bass_kernel_spmd(
        nc, in_maps, core_ids=list(range(NCORES)), **kw)
    _cache["last_result"] = res

    out = np.concatenate([r["y"] for r in res.results], axis=0)  # [256, 48, 512]
    return out
